# revision 23
# baseline (speedup 1.0000x reference)
"""Trainium2 Bass kernel for cross+self attention (dense_transformer).

Sharding: 8 cores = 2 (batch) x 4 (head-groups of 4 heads).
Each core computes, for its (b, hg):
  qkv projections for its 4 heads (tensor-parallel column split),
  kv projections of the context, rmsnorm(q), rmsnorm(k),
  softmax(q k^T / sqrt(d)) @ v, and a row-split partial of the output
  projection. Host sums the 4 partial proj outputs per batch.

All device tensors keep contraction-on-partitions layouts:
  qT/kT: [d, n] (d on partitions), v: [m, d], S^T: [m, n].
Everything flows in bf16 (inputs, weights, activations) with f32 PSUM
accumulation.

Schedule (from trace analysis of earlier revisions):
  - PE pstate: the tensor engine only reaches full clock after ~3us of
    continuous execution; any stall resets it to half speed.  Phase 1 is
    emitted as one continuous PE stream: projection chains write RAW
    (unnormalized) bf16 outputs in place, and the rmsnorm finishes
    (sumsq matmul -> Ln -> Exp -> broadcast matmul -> in-place multiply)
    trail 2 slots behind so PE never waits on ACT/DVE results.
  - rmsnorm scale = exp(-0.5*ln(var)) on ACT: Ln+Exp live in the same
    activation table as softmax's Exp -> one table load for the whole
    kernel, and no slow DVE reciprocal on the critical path.
  - context-side attention for n-blocks 0..NBE-1 (QK^T + exp into e2c)
    runs inside phase 1, stealing otherwise-idle ACT cycles; the last
    block computes its context logits inline in phase 2.
  - phase 2 PSUM budget (8 banks): s2 2x[128,2x512] (4) + av [65,2x512]
    (2) + rcb (1) + proj (1).  The av accumulator pair is single
    buffered; its norm/reciprocal consumers are emitted at steps 0-1 of
    the NEXT unit, before that unit's first AV matmul, so the
    write-after-read hazard never stalls PE.
  - DMA dispatches run on the sync HWDGE queue during compute: a
    dispatch on the scalar queue (~0.7us) head-of-line blocks the exp
    stream that attention depends on.  Only the idle startup window and
    the final drain use the scalar queue as a second lane.
  - final-block proj PSUM->SBUF copies run on ACT (idle once the last
    exp retires) so the tail drain never waits on the DVE queue.
  - weights/activations land as 4 chunk-pair tiles each so the first
    matmul chains only depend on the chunks they actually read.
  - softmax denominators (appended ones-column of v) inverted with the
    single-instruction DVE reciprocal_approx_fast; custom-DVE ops
    misread large-magnitude PSUM values, so stage via SBUF.
  - softmax never needs max-subtraction: rmsnormed q,k scaled by
    1/sqrt(d) keep logits ~N(0,1); fp32/bf16 exp cannot overflow.
"""

import sys
import numpy as np

if '/opt/trn_rl_repo' not in sys.path:
    sys.path.insert(0, '/opt/trn_rl_repo')

import concourse.bacc as bacc
import concourse.mybir as mybir
import concourse.tile as tile
from concourse.bass_utils import run_bass_kernel_spmd

f32 = mybir.dt.float32
f32r = mybir.dt.float32r
bf16 = mybir.dt.bfloat16
AF = mybir.ActivationFunctionType

# problem shapes (hardcoded per contract)
DIM = 1024
HEADS = 16
D = 64
B = 2
N = 2048
M = 512
EPS = 1e-6
SCALE = D ** -0.5  # 0.125

P = 128
CC = DIM // P          # 8 contraction chunks
NCP = CC // 2          # chunk pairs per tensor
HG = 4                 # heads per core
NPAIR = 2              # head pairs per core
NB = 512               # n-block width
NNB = N // NB          # 4 n-blocks
MT = N + M             # 2560 total kv length
NMC = MT // P          # 20 m-chunks (0..15 from x, 16..19 from context)
NMCX = N // P          # 16 m-chunks from x
NMCC = M // P          # 4 m-chunks from context
NBE = 3                # n-blocks whose ctx attention is precomputed (e2c)

_cached = None


def _build_module():
    nc = bacc.Bacc("TRN2", target_bir_lowering=False, debug=False, num_devices=8)

    xT_d = nc.dram_tensor("xT", [DIM, N], bf16, kind="ExternalInput").ap()
    ctxT_d = nc.dram_tensor("ctxT", [DIM, M], bf16, kind="ExternalInput").ap()
    wqkvT_d = nc.dram_tensor("wqkvT", [DIM, 3 * HG * D], bf16, kind="ExternalInput").ap()
    wkvyT_d = nc.dram_tensor("wkvyT", [DIM, 2 * HG * D], bf16, kind="ExternalInput").ap()
    wpT_d = nc.dram_tensor("wpT", [HG * D, DIM], bf16, kind="ExternalInput").ap()
    indsum_d = nc.dram_tensor("indsum", [P, 2 * (D + 1)], bf16, kind="ExternalInput").ap()
    ones64_d = nc.dram_tensor("ones64", [1, D], f32r, kind="ExternalInput").ap()
    ones64b_d = nc.dram_tensor("ones64b", [1, D], bf16, kind="ExternalInput").ap()
    out_d = nc.dram_tensor("out", [N, DIM], f32, kind="ExternalOutput").ap()

    with tile.TileContext(nc) as tc:
        _emit(nc, tc, xT_d, ctxT_d, wqkvT_d, wkvyT_d, wpT_d,
              indsum_d, ones64_d, ones64b_d, out_d)
    nc.compile()
    return nc


def _emit(nc, tc, xT_d, ctxT_d, wqkvT_d, wkvyT_d, wpT_d,
          indsum_d, ones64_d, ones64b_d, out_d):
    from collections import deque

    with (
        tc.tile_pool(name="live", bufs=1) as live,
        tc.tile_pool(name="work", bufs=3) as work,
        tc.tile_pool(name="rsp", bufs=4) as rsp,
        tc.tile_pool(name="epool", bufs=4) as epool,
        tc.tile_pool(name="xp", bufs=2) as xp,
    ):
        # ---- long-lived tiles ----
        indsum = live.tile([P, 2, D + 1], bf16)
        ones64 = live.tile([1, D], f32r)
        ones64b = live.tile([1, D], bf16)
        eps_t = live.tile([D + 1, 1], f32)
        qTn = [live.tile([P, N], bf16, tag=f"qTn{p}", name=f"qTn{p}")
               for p in range(NPAIR)]
        kTn = [live.tile([P, MT], bf16, tag=f"kTn{p}", name=f"kTn{p}")
               for p in range(NPAIR)]
        vA = live.tile([P, NMC, HG, D + 1], bf16)
        outn = [live.tile([P, N], bf16, tag=f"outn{p}", name=f"outn{p}")
                for p in range(NPAIR)]
        # ctx-attention exp(S^T) for n-blocks 0..NBE-1, computed in phase 1
        e2c = live.tile([P, NBE, NPAIR, NMCC, 2, NB], bf16)
        # weights/activations as chunk-pair tiles for fine DMA dependencies
        wqkv = [live.tile([P, 2, 3 * HG * D], bf16, tag=f"wqkv{i}",
                          name=f"wqkv{i}") for i in range(NCP)]
        wkvy = [live.tile([P, 2, 2 * HG * D], bf16, tag=f"wkvy{i}",
                          name=f"wkvy{i}") for i in range(NCP)]
        ctxT = [live.tile([P, 2, M], bf16, tag=f"ctxT{i}",
                          name=f"ctxT{i}") for i in range(NCP)]
        wpT = live.tile([P, NPAIR, DIM], bf16)

        nc.vector.memset(eps_t[:], EPS)
        nc.vector.memset(vA[:, :, :, D:D + 1], 1.0)

        # ---- startup DMA schedule ----
        # the first chains (ky/vy) need ctxT+wkvy; q/kx/vx need wqkv+xq.
        # Two HWDGE queues (sync, scalar) are both idle here; first-consumed
        # chunks dispatched first and finest.
        wkvy_r = wkvyT_d.rearrange("(o p) c -> p o c", p=P)
        wqkv_r = wqkvT_d.rearrange("(o p) c -> p o c", p=P)
        ctxT_r = ctxT_d.rearrange("(o p) m -> p o m", p=P)
        xT_r = xT_d.rearrange("(o p) n -> p o n", p=P)
        wpT_r = wpT_d.rearrange("(o p) c -> p o c", p=P)

        # ctx chunks land via sync, wkvy via scalar: the first chain's
        # operands arrive in parallel.  The ACT table pin waits until after
        # the critical scalar-queue dispatches.
        nc.sync.dma_start(ctxT[0][:, 0, 0:M // 2], ctxT_r[:, 0, 0:M // 2])
        nc.scalar.dma_start(wkvy[0][:, 0, 0:HG * D], wkvy_r[:, 0, 0:HG * D])
        nc.sync.dma_start(ctxT[0][:, 0, M // 2:], ctxT_r[:, 0, M // 2:])
        nc.scalar.dma_start(wkvy[0][:, 0, HG * D:], wkvy_r[:, 0, HG * D:])
        nc.sync.dma_start(ctxT[0][:, 1, :], ctxT_r[:, 1, :])
        nc.scalar.dma_start(wkvy[0][:, 1, :], wkvy_r[:, 1, :])
        for i in range(1, NCP):
            nc.sync.dma_start(ctxT[i][:], ctxT_r[:, 2 * i:2 * i + 2, :])
            nc.scalar.dma_start(wkvy[i][:], wkvy_r[:, 2 * i:2 * i + 2, :])
        # pre-pin the activation table containing BOTH Ln and Exp: the
        # automatic table-load pass greedily alternates tables (one 1.3us
        # ACT_TABLE_LOAD per activation otherwise)
        from concourse.hw_specs import get_activation_tables
        _tabs = get_activation_tables(nc.m.arch)
        _set_id = list(_tabs.keys()).index('natural_log_exp_and_others')
        nc.scalar.add_instruction(mybir.InstLoadActFuncSet(
            name=nc.get_next_instruction_name(), ins=[], outs=[],
            act_func_set_id=_set_id))
        nc.scalar.dma_start(indsum[:],
                            indsum_d.rearrange("p (a b) -> p a b", a=2))
        nc.scalar.dma_start(ones64[:], ones64_d[:])
        nc.scalar.dma_start(ones64b[:], ones64b_d[:])
        for i in range(NCP):
            nc.scalar.dma_start(wqkv[i][:], wqkv_r[:, 2 * i:2 * i + 2, :])

        xq_tiles = {}

        def fetch_x_block(nb, eng):
            xq = [xp.tile([P, 2, NB], bf16, tag=f"xq{i}", name=f"xq{i}")
                  for i in range(NCP)]
            xq_tiles[nb] = xq
            for i in range(NCP):
                eng.dma_start(xq[i][:], xT_r[:, 2 * i:2 * i + 2,
                                             nb * NB:(nb + 1) * NB])

        fetch_x_block(0, nc.sync)

        # ---- phase 1: projections + rmsnorm + precomputed ctx attention ----
        # Deferred-finish pipeline: at the START of each chain slot, the
        # sumsq finish of the chain 2 slots back and the normalize finish 2
        # slots after that are emitted, so every PE instruction's inputs were
        # produced >=1 full chain earlier and PE streams without stalling.
        slot_i = [0]
        fin2q = deque()   # (slot, fn) -> emits sumsq mm + Ln + Exp
        fin4q = deque()   # (slot, fn) -> emits broadcast mm + in-place mul
        hooks = deque()   # e2c ctx-attention units

        with (
            tc.tile_pool(name="ps1", bufs=2, space="PSUM") as ps1,
            tc.tile_pool(name="ps1s", bufs=1, space="PSUM") as ps1s,
            tc.tile_pool(name="ps1r", bufs=1, space="PSUM") as ps1r,
            tc.tile_pool(name="cs2", bufs=2, space="PSUM") as cs2p,
        ):
            def start_slot():
                slot_i[0] += 1
                s = slot_i[0]
                if fin2q and s - fin2q[0][0] >= 2:
                    _, fn = fin2q.popleft()
                    fin4q.append((s, fn()))
                if fin4q and s - fin4q[0][0] >= 2:
                    fin4q.popleft()[1]()
                if hooks:
                    hooks.popleft()()

            def norm_chain(w_tiles, w_col0, rhs_tiles, width, qk, dst,
                           dst_sl):
                """dst[:, dst_sl] = per-head rmsnorm of a [128, width]
                projection chunk (2 heads on partition halves).  The raw
                projection is written in place; the normalize multiply is
                deferred via fin2q/fin4q."""
                start_slot()
                acc = ps1.tile([P, NB], f32, tag="acc", name="acc")
                for ci in range(CC):
                    nc.tensor.matmul(
                        acc[:, :width],
                        w_tiles[ci // 2][:, ci % 2, w_col0:w_col0 + P],
                        rhs_tiles[ci // 2][:, ci % 2, 0:width],
                        start=(ci == 0), stop=(ci == CC - 1))
                nc.vector.tensor_copy(dst[:, dst_sl], acc[:, :width])
                sq = work.tile([P, NB], bf16, tag="sq", name="sq")
                nc.vector.tensor_mul(sq[:, :width], dst[:, dst_sl],
                                     dst[:, dst_sl])

                def fin2():
                    # sumsq lands on partitions 0 and 64: engine APs must
                    # start at a quad-aligned partition, so the two heads'
                    # scales stay addressable for the Exp/broadcast below
                    ms = ps1s.tile([D + 1, NB], f32, tag="ms", name="ms")
                    nc.tensor.matmul(ms[:, :width], indsum[:, qk, :],
                                     sq[:, :width], start=True, stop=True)
                    lnv = work.tile([D + 1, NB], f32, tag="lnv", name="lnv")
                    nc.scalar.activation(lnv[:, :width], ms[:, :width], AF.Ln,
                                         scale=1.0 / D, bias=eps_t[:])
                    rs0 = rsp.tile([1, NB], bf16, tag="rs0", name="rs0")
                    nc.scalar.activation(rs0[:, :width], lnv[0:1, :width],
                                         AF.Exp, scale=-0.5)
                    rs1 = rsp.tile([1, NB], bf16, tag="rs1", name="rs1")
                    nc.scalar.activation(rs1[:, :width], lnv[D:D + 1, :width],
                                         AF.Exp, scale=-0.5)

                    def fin4():
                        # broadcast each head's per-n scale over its 64 d
                        # partitions with a contraction-1 PE matmul (ones
                        # stationary; output base partition picks the PE
                        # quadrant), then one in-place DVE multiply.  The
                        # rmsnorm weights are folded into the projection
                        # weights on the host (indsum carries 1/w^2).
                        rsb = ps1r.tile([P, NB], f32, tag="rsb", name="rsb")
                        nc.tensor.matmul(rsb[0:D, :width], ones64b[:],
                                         rs0[:, :width], start=True,
                                         stop=True)
                        nc.tensor.matmul(rsb[D:P, :width], ones64b[:],
                                         rs1[:, :width], start=True,
                                         stop=True)
                        nc.vector.tensor_mul(dst[:, dst_sl], dst[:, dst_sl],
                                             rsb[:, :width])
                    return fin4

                fin2q.append((slot_i[0], fin2))

            def v_chain(src_tiles, src_col0, w_tiles, w_col0, mc_global):
                start_slot()
                acc = ps1.tile([P, NB], f32, tag="acc", name="acc")
                for ci in range(CC):
                    nc.tensor.matmul(
                        acc[:, :HG * D],
                        src_tiles[ci // 2][:, ci % 2, src_col0:src_col0 + P],
                        w_tiles[ci // 2][:, ci % 2, w_col0:w_col0 + HG * D],
                        start=(ci == 0), stop=(ci == CC - 1))
                nc.vector.tensor_copy(
                    vA[:, mc_global, :, 0:D],
                    acc[:, :HG * D].rearrange("p (h d) -> p h d", d=D))

            def make_ctx_unit(nb, p, mcc):
                """QK^T + exp for one context kv chunk of n-block nb,
                stealing idle ACT time in the PE-bound projection phase."""
                def _unit():
                    nbs = slice(nb * NB, (nb + 1) * NB)
                    mc = NMCX + mcc
                    s2 = cs2p.tile([P, 2, NB], f32, tag="cs2", name="cs2")
                    nc.tensor.matmul(
                        s2[:, 0, :], kTn[p][0:D, mc * P:(mc + 1) * P],
                        qTn[p][0:D, nbs], start=True, stop=True)
                    nc.tensor.matmul(
                        s2[:, 1, :], kTn[p][D:P, mc * P:(mc + 1) * P],
                        qTn[p][D:P, nbs], start=True, stop=True)
                    nc.scalar.activation(
                        e2c[:, nb, p, mcc, :, :].rearrange("p a b -> p (a b)"),
                        s2[:].rearrange("p a b -> p (a b)"),
                        AF.Exp, scale=SCALE)
                return _unit

            # context chains first: their finishes are needed by the e2c
            # units; vy chains fill the slots between for finish lag.
            norm_chain(wkvy, 0, ctxT, M, 1, kTn[0], slice(N, N + M))
            v_chain(ctxT, 0, wkvy, HG * D, NMCX + 0)
            norm_chain(wkvy, P, ctxT, M, 1, kTn[1], slice(N, N + M))
            v_chain(ctxT, P, wkvy, HG * D, NMCX + 1)
            v_chain(ctxT, 2 * P, wkvy, HG * D, NMCX + 2)
            v_chain(ctxT, 3 * P, wkvy, HG * D, NMCX + 3)

            for nb in range(NNB):
                if nb + 1 < NNB:
                    fetch_x_block(nb + 1, nc.sync)
                if nb == 1:
                    for pr in range(NPAIR):
                        nc.sync.dma_start(wpT[:, pr, :], wpT_r[:, pr, :])
                xq = xq_tiles.pop(nb)
                nbs = slice(nb * NB, (nb + 1) * NB)
                norm_chain(wqkv, 0, xq, NB, 0, qTn[0], nbs)
                norm_chain(wqkv, P, xq, NB, 0, qTn[1], nbs)
                v_chain(xq, 0, wqkv, 2 * HG * D, nb * (NB // P) + 0)
                norm_chain(wqkv, HG * D, xq, NB, 1, kTn[0], nbs)
                v_chain(xq, P, wqkv, 2 * HG * D, nb * (NB // P) + 1)
                norm_chain(wqkv, HG * D + P, xq, NB, 1, kTn[1], nbs)
                v_chain(xq, 2 * P, wqkv, 2 * HG * D, nb * (NB // P) + 2)
                v_chain(xq, 3 * P, wqkv, 2 * HG * D, nb * (NB // P) + 3)
                # q(nb) + ky normalizes are emitted by now (lag 4 slots);
                # queue this block's ctx attention for the following slots
                if nb < NBE:
                    for p in range(NPAIR):
                        for mcc in range(NMCC):
                            hooks.append(make_ctx_unit(nb, p, mcc))

            while fin2q or fin4q or hooks:
                start_slot()

        # ---- phase 2: attention + proj, per (n-block, pair) unit ----
        # AV matmuls are emitted 2 steps behind their exp so the PE stream
        # never head-of-line blocks on ACT.  The av accumulator is single
        # buffered: its readers (norm units) run at steps 0-1 of the next
        # unit, before that unit's first AV matmul is emitted.
        pend_av = deque()
        norm_q = deque()
        tail_q = deque()

        with (
            tc.tile_pool(name="ps2s", bufs=2, space="PSUM") as ps2s,
            tc.tile_pool(name="ps2av", bufs=1, space="PSUM") as ps2av,
            tc.tile_pool(name="ps2x", bufs=2, space="PSUM") as ps2x,
        ):
            def emit_av(item):
                av, p_, si, mc_, e_tt = item
                for j in range(2):
                    nc.tensor.matmul(av[:, j, :], vA[:, mc_, 2 * p_ + j, :],
                                     e_tt[:, j, :],
                                     start=(si == 0), stop=(si == NMC - 1))

            def make_norm_pair(p, nb, av):
                """outn rows for both heads.  All four av reads are emitted
                first (step 0) so the next unit's first AV matmul clears the
                single-buffered accumulator's write-after-read hazard as
                early as possible; the reciprocal/broadcast/multiply chain
                follows at step 1."""
                nbs = slice(nb * NB, (nb + 1) * NB)
                st = {}

                def _reads():
                    for j in range(2):
                        den = work.tile([1, NB], f32, tag="den",
                                        name="den")
                        nc.vector.tensor_copy(den[:], av[D:D + 1, j, :])
                        avn = work.tile([D, NB], bf16, tag="avn",
                                        name="avn")
                        nc.vector.tensor_copy(avn[:], av[0:D, j, :])
                        st[j] = (den, avn)

                def _compute():
                    for j in range(2):
                        den, avn = st[j]
                        rc1 = work.tile([1, NB], f32, tag="rc1",
                                        name="rc1")
                        nc.vector.reciprocal_approx_fast(out=rc1[:],
                                                         in_=den[:])
                        rc1b = work.tile([1, NB], f32r, tag="rc1b",
                                         name="rc1b")
                        with nc.allow_low_precision(reason="fp22 matmul in"):
                            nc.vector.tensor_copy(rc1b[:], rc1[:])
                        # rcb shares the proj ring: norm (steps 0-1)
                        # and proj (steps 6+) never hold it simultaneously
                        rcbt = ps2x.tile([P, NB], f32, tag="px", name="rcbt")
                        rcb = rcbt[0:D, :]
                        nc.tensor.matmul(rcb[:], ones64[:], rc1b[:],
                                         start=True, stop=True)
                        nc.vector.tensor_mul(
                            outn[p][j * D:(j + 1) * D, nbs], avn[:], rcb[:])
                return _reads, _compute

            def make_proj_unit(nb, nch, co):
                def _proj():
                    n0 = nb * NB + nch * P
                    pp = ps2x.tile([P, NB], f32, tag="px", name="pp")
                    for pr in range(NPAIR):
                        nc.tensor.matmul(pp[:], outn[pr][:, n0:n0 + P],
                                         wpT[:, pr, co * NB:(co + 1) * NB],
                                         start=(pr == 0),
                                         stop=(pr == NPAIR - 1))
                    po = work.tile([P, NB], f32, tag="po", name="po")
                    if nb == NNB - 1 and (nch * 2 + co) % 2 == 0:
                        # tail: ACT is idle once the last exp retires;
                        # alternate po copies over ACT and DVE so the
                        # drain pipelines across both engines
                        nc.scalar.activation(po[:], pp[:], AF.Copy)
                    else:
                        nc.vector.tensor_copy(po[:], pp[:])
                    if nb == NNB - 1:
                        # final block: split the writeback so the last
                        # transfers drain on both queues
                        h = NB // 2
                        nc.sync.dma_start(
                            out_d[n0:n0 + P, co * NB:co * NB + h],
                            po[:, 0:h])
                        nc.scalar.dma_start(
                            out_d[n0:n0 + P, co * NB + h:(co + 1) * NB],
                            po[:, h:])
                    else:
                        nc.sync.dma_start(
                            out_d[n0:n0 + P, co * NB:(co + 1) * NB], po[:])
                return _proj

            for nb in range(NNB):
                nbs = slice(nb * NB, (nb + 1) * NB)
                for p in range(NPAIR):
                    av = ps2av.tile([D + 1, 2, NB], f32, tag="av", name="av")
                    # ctx steps (ACT-free for e2c blocks) are interleaved
                    # so the exp deficit (ACT ~1.0us vs PE ~0.88us per x-step)
                    # resets before it can stall the s2 ring; one ctx step
                    # stays last so the end-of-unit AV drain is exp-free.
                    xs = iter(range(NMCX))
                    cs = iter(range(NMCX, NMC))
                    seq = [next(cs) if si in (0, 6, 13, 19) else next(xs)
                           for si in range(NMC)]
                    for si, mc in enumerate(seq):
                        if nb < NBE and mc >= NMCX:
                            e_t = e2c[:, nb, p, mc - NMCX, :, :]
                        else:
                            s2 = ps2s.tile([P, 2, NB], f32, tag="s2",
                                           name="s2")
                            nc.tensor.matmul(
                                s2[:, 0, :],
                                kTn[p][0:D, mc * P:(mc + 1) * P],
                                qTn[p][0:D, nbs], start=True, stop=True)
                            nc.tensor.matmul(
                                s2[:, 1, :],
                                kTn[p][D:P, mc * P:(mc + 1) * P],
                                qTn[p][D:P, nbs], start=True, stop=True)
                            e_t = epool.tile([P, 2, NB], bf16, tag="E",
                                             name="e_t")
                            nc.scalar.activation(
                                e_t[:].rearrange("p a b -> p (a b)"),
                                s2[:].rearrange("p a b -> p (a b)"),
                                AF.Exp, scale=SCALE)
                        if si < 2 and norm_q:
                            norm_q.popleft()()
                        elif si >= 6 and tail_q:
                            tail_q.popleft()()
                        pend_av.append((av, p, si, mc, e_t))
                        if len(pend_av) > 2:
                            emit_av(pend_av.popleft())
                    while pend_av:
                        emit_av(pend_av.popleft())
                    norm_q.extend(make_norm_pair(p, nb, av))
                for nch in range(NB // P):
                    for co in range(2):
                        tail_q.append(make_proj_unit(nb, nch, co))
            while norm_q:
                norm_q.popleft()()
            while tail_q:
                tail_q.popleft()()


def _get_module():
    global _cached
    if _cached is None:
        _cached = _build_module()
    return _cached


def _make_in_maps(x, context, qkv_w, kv_y_w, proj_w, q_norm_w, k_norm_w):
    import ml_dtypes
    b16 = ml_dtypes.bfloat16
    GD = HG * D  # 256 head-dims per core
    # rmsnorm weights are folded into the q/k projection weights; the
    # sumsq stationary then divides by w^2 so the variance is still that of
    # the raw projection.  (w==0 rows are zero either way.)
    qw = np.where(q_norm_w == 0, 1.0, q_norm_w).astype(np.float32)
    kw = np.where(k_norm_w == 0, 1.0, k_norm_w).astype(np.float32)
    indsum = np.zeros((P, 2, D + 1), np.float32)
    indsum[0:D, 0, 0] = 1.0 / qw ** 2
    indsum[D:P, 0, D] = 1.0 / qw ** 2
    indsum[0:D, 1, 0] = 1.0 / kw ** 2
    indsum[D:P, 1, D] = 1.0 / kw ** 2
    indsum = indsum.reshape(P, 2 * (D + 1))
    ones64 = np.ones((1, D), np.float32)
    qw_t = np.tile(q_norm_w, HG)[:, None].astype(np.float32)
    kw_t = np.tile(k_norm_w, HG)[:, None].astype(np.float32)
    projT = np.ascontiguousarray(proj_w.T)  # [ci, co]

    xT = [np.ascontiguousarray(x[b].T).astype(b16) for b in range(B)]
    cT = [np.ascontiguousarray(context[b].T).astype(b16) for b in range(B)]

    in_maps = []
    for core in range(8):
        b, hg = divmod(core, 4)
        r0 = hg * GD
        wq = qkv_w[r0:r0 + GD] * qw_t
        wk = qkv_w[DIM + r0:DIM + r0 + GD] * kw_t
        wv = qkv_w[2 * DIM + r0:2 * DIM + r0 + GD]
        wky = kv_y_w[r0:r0 + GD] * kw_t
        wvy = kv_y_w[DIM + r0:DIM + r0 + GD]
        in_maps.append({
            "xT": xT[b],
            "ctxT": cT[b],
            "wqkvT": np.ascontiguousarray(
                np.concatenate([wq, wk, wv], 0).T).astype(b16),
            "wkvyT": np.ascontiguousarray(
                np.concatenate([wky, wvy], 0).T).astype(b16),
            "wpT": np.ascontiguousarray(projT[r0:r0 + GD]).astype(b16),
            "indsum": indsum.astype(b16),
            "ones64": ones64,
            "ones64b": ones64.astype(b16),
        })
    return in_maps


def kernel(x, context, qkv_w, kv_y_w, proj_w, proj_b, q_norm_w, k_norm_w):
    x = np.asarray(x, np.float32)
    context = np.asarray(context, np.float32)
    qkv_w = np.asarray(qkv_w, np.float32)
    kv_y_w = np.asarray(kv_y_w, np.float32)
    proj_w = np.asarray(proj_w, np.float32)
    proj_b = np.asarray(proj_b, np.float32)
    q_norm_w = np.asarray(q_norm_w, np.float32)
    k_norm_w = np.asarray(k_norm_w, np.float32)

    nc = _get_module()
    in_maps = _make_in_maps(x, context, qkv_w, kv_y_w, proj_w,
                            q_norm_w, k_norm_w)
    res = run_bass_kernel_spmd(nc, in_maps, core_ids=list(range(8)))
    out = np.zeros((B, N, DIM), np.float32)
    for core in range(8):
        b = core // 4
        out[b] += res.results[core]["out"]
    out += proj_b[None, None, :]
    return out


# revision 24
# speedup vs baseline: 1.1540x; 1.1540x over previous
"""Trainium2 Bass kernel for cross+self attention (dense_transformer).

Sharding: 8 cores = 2 (batch) x 4 (head-groups of 4 heads).
Each core computes, for its (b, hg):
  qkv projections for its 4 heads (tensor-parallel column split),
  kv projections of the context, rmsnorm(q), rmsnorm(k),
  softmax(q k^T / sqrt(d)) @ v, and a row-split partial of the output
  projection. Host sums the 4 partial proj outputs per batch.

All device tensors keep contraction-on-partitions layouts:
  qT/kT: [d, n] (d on partitions), v: [m, d], S^T: [m, n].
Everything flows in bf16 (inputs, weights, activations) with f32 PSUM
accumulation.

Schedule (from trace analysis of earlier revisions):
  - PE pstate: the tensor engine only reaches full clock after ~3us of
    continuous execution; any stall resets it to half speed.  Phase 1 is
    emitted as one continuous PE stream: projection chains write RAW
    (unnormalized) bf16 outputs in place, and the rmsnorm finishes
    (sumsq matmul -> Ln -> Exp -> broadcast matmul -> in-place multiply)
    trail 2 slots behind so PE never waits on ACT/DVE results.
  - rmsnorm scale = exp(-0.5*ln(var)) on ACT: Ln+Exp live in the same
    activation table as softmax's Exp -> one table load for the whole
    kernel, and no slow DVE reciprocal on the critical path.
  - context-side attention for n-blocks 0..NBE-1 (QK^T + exp into e2c)
    runs inside phase 1, stealing otherwise-idle ACT cycles; the last
    block computes its context logits inline in phase 2.
  - phase 2 PSUM budget (8 banks): s2 2x[128,2x512] (4) + av [65,2x512]
    (2) + rcb (1) + proj (1).  The av accumulator pair is single
    buffered; its norm/reciprocal consumers are emitted at steps 0-1 of
    the NEXT unit, before that unit's first AV matmul, so the
    write-after-read hazard never stalls PE.
  - DMA dispatches run on the sync HWDGE queue during compute: a
    dispatch on the scalar queue (~0.7us) head-of-line blocks the exp
    stream that attention depends on.  Only the idle startup window and
    the final drain use the scalar queue as a second lane.
  - final-block proj PSUM->SBUF copies run on ACT (idle once the last
    exp retires) so the tail drain never waits on the DVE queue.
  - weights/activations land as 4 chunk-pair tiles each so the first
    matmul chains only depend on the chunks they actually read.
  - softmax denominators (appended ones-column of v) inverted with the
    single-instruction DVE reciprocal_approx_fast; custom-DVE ops
    misread large-magnitude PSUM values, so stage via SBUF.
  - softmax never needs max-subtraction: rmsnormed q,k scaled by
    1/sqrt(d) keep logits ~N(0,1); fp32/bf16 exp cannot overflow.
"""

import sys
import numpy as np

if '/opt/trn_rl_repo' not in sys.path:
    sys.path.insert(0, '/opt/trn_rl_repo')

import concourse.bacc as bacc
import concourse.mybir as mybir
import concourse.tile as tile
from concourse.bass_utils import run_bass_kernel_spmd

f32 = mybir.dt.float32
f32r = mybir.dt.float32r
bf16 = mybir.dt.bfloat16
AF = mybir.ActivationFunctionType

# problem shapes (hardcoded per contract)
DIM = 1024
HEADS = 16
D = 64
B = 2
N = 2048
M = 512
EPS = 1e-6
SCALE = D ** -0.5  # 0.125

P = 128
CC = DIM // P          # 8 contraction chunks
NCP = CC // 2          # chunk pairs per tensor
HG = 4                 # heads per core
NPAIR = 2              # head pairs per core
NB = 512               # n-block width
NNB = N // NB          # 4 n-blocks
MT = N + M             # 2560 total kv length
NMC = MT // P          # 20 m-chunks (0..15 from x, 16..19 from context)
NMCX = N // P          # 16 m-chunks from x
NMCC = M // P          # 4 m-chunks from context
NBE = 3                # n-blocks whose ctx attention is precomputed (e2c)

_cached = None


def _build_module():
    nc = bacc.Bacc("TRN2", target_bir_lowering=False, debug=False, num_devices=8)

    xT_d = nc.dram_tensor("xT", [DIM, N], bf16, kind="ExternalInput").ap()
    ctxT_d = nc.dram_tensor("ctxT", [DIM, M], bf16, kind="ExternalInput").ap()
    wqkvT_d = nc.dram_tensor("wqkvT", [DIM, 3 * HG * D], bf16, kind="ExternalInput").ap()
    wkvyT_d = nc.dram_tensor("wkvyT", [DIM, 2 * HG * D], bf16, kind="ExternalInput").ap()
    wpT_d = nc.dram_tensor("wpT", [HG * D, DIM], bf16, kind="ExternalInput").ap()
    indsum_d = nc.dram_tensor("indsum", [P, 2 * (D + 1)], bf16, kind="ExternalInput").ap()
    ones64_d = nc.dram_tensor("ones64", [1, D], f32r, kind="ExternalInput").ap()
    ones64b_d = nc.dram_tensor("ones64b", [1, D], bf16, kind="ExternalInput").ap()
    out_d = nc.dram_tensor("out", [N, DIM], f32, kind="ExternalOutput").ap()

    with tile.TileContext(nc) as tc:
        _emit(nc, tc, xT_d, ctxT_d, wqkvT_d, wkvyT_d, wpT_d,
              indsum_d, ones64_d, ones64b_d, out_d)
    nc.compile()
    return nc


def _emit(nc, tc, xT_d, ctxT_d, wqkvT_d, wkvyT_d, wpT_d,
          indsum_d, ones64_d, ones64b_d, out_d):
    from collections import deque

    with (
        tc.tile_pool(name="live", bufs=1) as live,
        tc.tile_pool(name="work", bufs=3) as work,
        tc.tile_pool(name="rsp", bufs=4) as rsp,
        tc.tile_pool(name="epool", bufs=4) as epool,
        tc.tile_pool(name="xp", bufs=2) as xp,
    ):
        # ---- long-lived tiles ----
        indsum = live.tile([P, 2, D + 1], bf16)
        ones64 = live.tile([1, D], f32r)
        ones64b = live.tile([1, D], bf16)
        eps_t = live.tile([D + 1, 1], f32)
        qTn = [live.tile([P, N], bf16, tag=f"qTn{p}", name=f"qTn{p}")
               for p in range(NPAIR)]
        kTn = [live.tile([P, MT], bf16, tag=f"kTn{p}", name=f"kTn{p}")
               for p in range(NPAIR)]
        vA = live.tile([P, NMC, HG, D + 1], bf16)
        outn = [live.tile([P, N], bf16, tag=f"outn{p}", name=f"outn{p}")
                for p in range(NPAIR)]
        # ctx-attention exp(S^T) for n-blocks 0..NBE-1, computed in phase 1
        e2c = live.tile([P, NBE, NPAIR, NMCC, 2, NB], bf16)
        # weights/activations as chunk-pair tiles for fine DMA dependencies
        wqkv = [live.tile([P, 2, 3 * HG * D], bf16, tag=f"wqkv{i}",
                          name=f"wqkv{i}") for i in range(NCP)]
        wkvy = [live.tile([P, 2, 2 * HG * D], bf16, tag=f"wkvy{i}",
                          name=f"wkvy{i}") for i in range(NCP)]
        ctxT = [live.tile([P, 2, M], bf16, tag=f"ctxT{i}",
                          name=f"ctxT{i}") for i in range(NCP)]
        wpT = live.tile([P, NPAIR, DIM], bf16)

        nc.vector.memset(eps_t[:], EPS)
        nc.vector.memset(vA[:, :, :, D:D + 1], 1.0)

        # ---- startup DMA schedule ----
        # the first chains (ky/vy) need ctxT+wkvy; q/kx/vx need wqkv+xq.
        # Two HWDGE queues (sync, scalar) are both idle here; first-consumed
        # chunks dispatched first and finest.
        wkvy_r = wkvyT_d.rearrange("(o p) c -> p o c", p=P)
        wqkv_r = wqkvT_d.rearrange("(o p) c -> p o c", p=P)
        ctxT_r = ctxT_d.rearrange("(o p) m -> p o m", p=P)
        xT_r = xT_d.rearrange("(o p) n -> p o n", p=P)
        wpT_r = wpT_d.rearrange("(o p) c -> p o c", p=P)

        # ctx chunks land via sync, wkvy via scalar: the first chain's
        # operands arrive in parallel.  The ACT table pin waits until after
        # the critical scalar-queue dispatches.
        nc.sync.dma_start(ctxT[0][:, 0, 0:M // 2], ctxT_r[:, 0, 0:M // 2])
        nc.scalar.dma_start(wkvy[0][:, 0, 0:HG * D], wkvy_r[:, 0, 0:HG * D])
        nc.sync.dma_start(ctxT[0][:, 0, M // 2:], ctxT_r[:, 0, M // 2:])
        nc.scalar.dma_start(wkvy[0][:, 0, HG * D:], wkvy_r[:, 0, HG * D:])
        nc.sync.dma_start(ctxT[0][:, 1, :], ctxT_r[:, 1, :])
        nc.scalar.dma_start(wkvy[0][:, 1, :], wkvy_r[:, 1, :])
        for i in range(1, NCP):
            nc.sync.dma_start(ctxT[i][:], ctxT_r[:, 2 * i:2 * i + 2, :])
            nc.scalar.dma_start(wkvy[i][:], wkvy_r[:, 2 * i:2 * i + 2, :])
        # pre-pin the activation table containing BOTH Ln and Exp: the
        # automatic table-load pass greedily alternates tables (one 1.3us
        # ACT_TABLE_LOAD per activation otherwise)
        from concourse.hw_specs import get_activation_tables
        _tabs = get_activation_tables(nc.m.arch)
        _set_id = list(_tabs.keys()).index('natural_log_exp_and_others')
        nc.scalar.add_instruction(mybir.InstLoadActFuncSet(
            name=nc.get_next_instruction_name(), ins=[], outs=[],
            act_func_set_id=_set_id))
        nc.scalar.dma_start(indsum[:],
                            indsum_d.rearrange("p (a b) -> p a b", a=2))
        nc.scalar.dma_start(ones64[:], ones64_d[:])
        nc.scalar.dma_start(ones64b[:], ones64b_d[:])
        for i in range(NCP):
            nc.scalar.dma_start(wqkv[i][:], wqkv_r[:, 2 * i:2 * i + 2, :])

        xq_tiles = {}

        def fetch_x_block(nb, eng):
            xq = [xp.tile([P, 2, NB], bf16, tag=f"xq{i}", name=f"xq{i}")
                  for i in range(NCP)]
            xq_tiles[nb] = xq
            for i in range(NCP):
                eng.dma_start(xq[i][:], xT_r[:, 2 * i:2 * i + 2,
                                             nb * NB:(nb + 1) * NB])

        fetch_x_block(0, nc.scalar)

        # ---- phase 1: projections + rmsnorm + precomputed ctx attention ----
        # Deferred-finish pipeline: at the START of each chain slot, the
        # sumsq finish of the chain 2 slots back and the normalize finish 2
        # slots after that are emitted, so every PE instruction's inputs were
        # produced >=1 full chain earlier and PE streams without stalling.
        slot_i = [0]
        fin2q = deque()   # (slot, fn) -> emits sumsq mm + Ln + Exp
        fin4q = deque()   # (slot, fn) -> emits broadcast mm + in-place mul
        hooks = deque()   # e2c ctx-attention units

        with (
            tc.tile_pool(name="ps1", bufs=2, space="PSUM") as ps1,
            tc.tile_pool(name="ps1s", bufs=1, space="PSUM") as ps1s,
            tc.tile_pool(name="ps1r", bufs=1, space="PSUM") as ps1r,
            tc.tile_pool(name="cs2", bufs=2, space="PSUM") as cs2p,
        ):
            def start_slot():
                slot_i[0] += 1
                s = slot_i[0]
                if fin2q and s - fin2q[0][0] >= 2:
                    _, fn = fin2q.popleft()
                    fin4q.append((s, fn()))
                if fin4q and s - fin4q[0][0] >= 2:
                    fin4q.popleft()[1]()
                if hooks:
                    hooks.popleft()()

            def norm_chain(w_tiles, w_col0, rhs_tiles, width, qk, dst,
                           dst_sl):
                """dst[:, dst_sl] = per-head rmsnorm of a [128, width]
                projection chunk (2 heads on partition halves).  The raw
                projection is written in place; the normalize multiply is
                deferred via fin2q/fin4q."""
                start_slot()
                acc = ps1.tile([P, NB], f32, tag="acc", name="acc")
                for ci in range(CC):
                    nc.tensor.matmul(
                        acc[:, :width],
                        w_tiles[ci // 2][:, ci % 2, w_col0:w_col0 + P],
                        rhs_tiles[ci // 2][:, ci % 2, 0:width],
                        start=(ci == 0), stop=(ci == CC - 1))
                nc.vector.tensor_copy(dst[:, dst_sl], acc[:, :width])
                sq = work.tile([P, NB], bf16, tag="sq", name="sq")
                nc.vector.tensor_mul(sq[:, :width], dst[:, dst_sl],
                                     dst[:, dst_sl])

                def fin2():
                    # sumsq lands on partitions 0 and 64: engine APs must
                    # start at a quad-aligned partition, so the two heads'
                    # scales stay addressable for the Exp/broadcast below
                    ms = ps1s.tile([D + 1, NB], f32, tag="ms", name="ms")
                    nc.tensor.matmul(ms[:, :width], indsum[:, qk, :],
                                     sq[:, :width], start=True, stop=True)
                    lnv = work.tile([D + 1, NB], f32, tag="lnv", name="lnv")
                    nc.scalar.activation(lnv[:, :width], ms[:, :width], AF.Ln,
                                         scale=1.0 / D, bias=eps_t[:])
                    rs0 = rsp.tile([1, NB], bf16, tag="rs0", name="rs0")
                    nc.scalar.activation(rs0[:, :width], lnv[0:1, :width],
                                         AF.Exp, scale=-0.5)
                    rs1 = rsp.tile([1, NB], bf16, tag="rs1", name="rs1")
                    nc.scalar.activation(rs1[:, :width], lnv[D:D + 1, :width],
                                         AF.Exp, scale=-0.5)

                    def fin4():
                        # broadcast each head's per-n scale over its 64 d
                        # partitions with a contraction-1 PE matmul (ones
                        # stationary; output base partition picks the PE
                        # quadrant), then one in-place DVE multiply.  The
                        # rmsnorm weights are folded into the projection
                        # weights on the host (indsum carries 1/w^2).
                        rsb = ps1r.tile([P, NB], f32, tag="rsb", name="rsb")
                        nc.tensor.matmul(rsb[0:D, :width], ones64b[:],
                                         rs0[:, :width], start=True,
                                         stop=True)
                        nc.tensor.matmul(rsb[D:P, :width], ones64b[:],
                                         rs1[:, :width], start=True,
                                         stop=True)
                        nc.vector.tensor_mul(dst[:, dst_sl], dst[:, dst_sl],
                                             rsb[:, :width])
                    return fin4

                fin2q.append((slot_i[0], fin2))

            def v_chain(src_tiles, src_col0, w_tiles, w_col0, mc_global):
                start_slot()
                acc = ps1.tile([P, NB], f32, tag="acc", name="acc")
                for ci in range(CC):
                    nc.tensor.matmul(
                        acc[:, :HG * D],
                        src_tiles[ci // 2][:, ci % 2, src_col0:src_col0 + P],
                        w_tiles[ci // 2][:, ci % 2, w_col0:w_col0 + HG * D],
                        start=(ci == 0), stop=(ci == CC - 1))
                nc.vector.tensor_copy(
                    vA[:, mc_global, :, 0:D],
                    acc[:, :HG * D].rearrange("p (h d) -> p h d", d=D))

            def make_ctx_unit(nb, p, mcc):
                """QK^T + exp for one context kv chunk of n-block nb,
                stealing idle ACT time in the PE-bound projection phase."""
                def _unit():
                    nbs = slice(nb * NB, (nb + 1) * NB)
                    mc = NMCX + mcc
                    s2 = cs2p.tile([P, 2, NB], f32, tag="cs2", name="cs2")
                    nc.tensor.matmul(
                        s2[:, 0, :], kTn[p][0:D, mc * P:(mc + 1) * P],
                        qTn[p][0:D, nbs], start=True, stop=True)
                    nc.tensor.matmul(
                        s2[:, 1, :], kTn[p][D:P, mc * P:(mc + 1) * P],
                        qTn[p][D:P, nbs], start=True, stop=True)
                    nc.scalar.activation(
                        e2c[:, nb, p, mcc, :, :].rearrange("p a b -> p (a b)"),
                        s2[:].rearrange("p a b -> p (a b)"),
                        AF.Exp, scale=SCALE)
                return _unit

            # context chains first: their finishes are needed by the e2c
            # units; vy chains fill the slots between for finish lag.
            norm_chain(wkvy, 0, ctxT, M, 1, kTn[0], slice(N, N + M))
            v_chain(ctxT, 0, wkvy, HG * D, NMCX + 0)
            norm_chain(wkvy, P, ctxT, M, 1, kTn[1], slice(N, N + M))
            v_chain(ctxT, P, wkvy, HG * D, NMCX + 1)
            v_chain(ctxT, 2 * P, wkvy, HG * D, NMCX + 2)
            v_chain(ctxT, 3 * P, wkvy, HG * D, NMCX + 3)

            for nb in range(NNB):
                if nb + 1 < NNB:
                    fetch_x_block(nb + 1, nc.sync)
                if nb == 1:
                    for pr in range(NPAIR):
                        nc.sync.dma_start(wpT[:, pr, :], wpT_r[:, pr, :])
                xq = xq_tiles.pop(nb)
                nbs = slice(nb * NB, (nb + 1) * NB)
                norm_chain(wqkv, 0, xq, NB, 0, qTn[0], nbs)
                norm_chain(wqkv, P, xq, NB, 0, qTn[1], nbs)
                v_chain(xq, 0, wqkv, 2 * HG * D, nb * (NB // P) + 0)
                norm_chain(wqkv, HG * D, xq, NB, 1, kTn[0], nbs)
                v_chain(xq, P, wqkv, 2 * HG * D, nb * (NB // P) + 1)
                norm_chain(wqkv, HG * D + P, xq, NB, 1, kTn[1], nbs)
                v_chain(xq, 2 * P, wqkv, 2 * HG * D, nb * (NB // P) + 2)
                v_chain(xq, 3 * P, wqkv, 2 * HG * D, nb * (NB // P) + 3)
                # q(nb) + ky normalizes are emitted by now (lag 4 slots);
                # queue this block's ctx attention for the following slots
                if nb < NBE:
                    for p in range(NPAIR):
                        for mcc in range(NMCC):
                            hooks.append(make_ctx_unit(nb, p, mcc))

            while fin2q or fin4q or hooks:
                start_slot()

        # ---- phase 2: attention + proj, per (n-block, pair) unit ----
        # AV matmuls are emitted 2 steps behind their exp so the PE stream
        # never head-of-line blocks on ACT.  The av accumulator is single
        # buffered: its readers (norm units) run at steps 0-1 of the next
        # unit, before that unit's first AV matmul is emitted.
        pend_av = deque()
        norm_q = deque()
        tail_q = deque()

        with (
            tc.tile_pool(name="ps2s", bufs=2, space="PSUM") as ps2s,
            tc.tile_pool(name="ps2av", bufs=1, space="PSUM") as ps2av,
            tc.tile_pool(name="ps2x", bufs=2, space="PSUM") as ps2x,
        ):
            def emit_av(item):
                av, p_, si, mc_, e_tt = item
                for j in range(2):
                    nc.tensor.matmul(av[:, j, :], vA[:, mc_, 2 * p_ + j, :],
                                     e_tt[:, j, :],
                                     start=(si == 0), stop=(si == NMC - 1))

            def make_norm_pair(p, nb, av):
                """outn rows for both heads.  All four av reads are emitted
                first (step 0) so the next unit's first AV matmul clears the
                single-buffered accumulator's write-after-read hazard as
                early as possible; the reciprocal/broadcast/multiply chain
                follows at step 1."""
                nbs = slice(nb * NB, (nb + 1) * NB)
                st = {}

                def _reads():
                    for j in range(2):
                        den = work.tile([1, NB], f32, tag="den",
                                        name="den")
                        nc.vector.tensor_copy(den[:], av[D:D + 1, j, :])
                        avn = work.tile([D, NB], bf16, tag="avn",
                                        name="avn")
                        nc.vector.tensor_copy(avn[:], av[0:D, j, :])
                        st[j] = (den, avn)

                def _compute():
                    for j in range(2):
                        den, avn = st[j]
                        rc1 = work.tile([1, NB], f32, tag="rc1",
                                        name="rc1")
                        nc.vector.reciprocal_approx_fast(out=rc1[:],
                                                         in_=den[:])
                        rc1b = work.tile([1, NB], f32r, tag="rc1b",
                                         name="rc1b")
                        with nc.allow_low_precision(reason="fp22 matmul in"):
                            nc.vector.tensor_copy(rc1b[:], rc1[:])
                        # rcb shares the proj ring: norm (steps 0-1)
                        # and proj (steps 6+) never hold it simultaneously
                        rcbt = ps2x.tile([P, NB], f32, tag="px", name="rcbt")
                        rcb = rcbt[0:D, :]
                        nc.tensor.matmul(rcb[:], ones64[:], rc1b[:],
                                         start=True, stop=True)
                        nc.vector.tensor_mul(
                            outn[p][j * D:(j + 1) * D, nbs], avn[:], rcb[:])
                return _reads, _compute

            def make_proj_unit(nb, nch, co):
                def _proj():
                    n0 = nb * NB + nch * P
                    pp = ps2x.tile([P, NB], f32, tag="px", name="pp")
                    for pr in range(NPAIR):
                        nc.tensor.matmul(pp[:], outn[pr][:, n0:n0 + P],
                                         wpT[:, pr, co * NB:(co + 1) * NB],
                                         start=(pr == 0),
                                         stop=(pr == NPAIR - 1))
                    po = work.tile([P, NB], f32, tag="po", name="po")
                    if nb == NNB - 1 and (nch * 2 + co) % 2 == 0:
                        # tail: ACT is idle once the last exp retires;
                        # alternate po copies over ACT and DVE so the
                        # drain pipelines across both engines
                        nc.scalar.activation(po[:], pp[:], AF.Copy)
                    else:
                        nc.vector.tensor_copy(po[:], pp[:])
                    if nb == NNB - 1:
                        # final block: split the writeback so the last
                        # transfers drain on both queues
                        h = NB // 2
                        nc.sync.dma_start(
                            out_d[n0:n0 + P, co * NB:co * NB + h],
                            po[:, 0:h])
                        nc.scalar.dma_start(
                            out_d[n0:n0 + P, co * NB + h:(co + 1) * NB],
                            po[:, h:])
                    else:
                        nc.sync.dma_start(
                            out_d[n0:n0 + P, co * NB:(co + 1) * NB], po[:])
                return _proj

            for nb in range(NNB):
                nbs = slice(nb * NB, (nb + 1) * NB)
                for p in range(NPAIR):
                    av = ps2av.tile([D + 1, 2, NB], f32, tag="av", name="av")
                    # ctx steps (ACT-free for e2c blocks) are interleaved
                    # so the exp deficit (ACT ~1.0us vs PE ~0.88us per x-step)
                    # resets before it can stall the s2 ring; one ctx step
                    # stays last so the end-of-unit AV drain is exp-free.
                    xs = iter(range(NMCX))
                    cs = iter(range(NMCX, NMC))
                    seq = [next(cs) if si in (0, 6, 13, 19) else next(xs)
                           for si in range(NMC)]
                    for si, mc in enumerate(seq):
                        if nb < NBE and mc >= NMCX:
                            e_t = e2c[:, nb, p, mc - NMCX, :, :]
                        else:
                            s2 = ps2s.tile([P, 2, NB], f32, tag="s2",
                                           name="s2")
                            nc.tensor.matmul(
                                s2[:, 0, :],
                                kTn[p][0:D, mc * P:(mc + 1) * P],
                                qTn[p][0:D, nbs], start=True, stop=True)
                            nc.tensor.matmul(
                                s2[:, 1, :],
                                kTn[p][D:P, mc * P:(mc + 1) * P],
                                qTn[p][D:P, nbs], start=True, stop=True)
                            e_t = epool.tile([P, 2, NB], bf16, tag="E",
                                             name="e_t")
                            nc.scalar.activation(
                                e_t[:].rearrange("p a b -> p (a b)"),
                                s2[:].rearrange("p a b -> p (a b)"),
                                AF.Exp, scale=SCALE)
                        if si < 2 and norm_q:
                            norm_q.popleft()()
                        elif si >= 6 and tail_q:
                            tail_q.popleft()()
                        pend_av.append((av, p, si, mc, e_t))
                        if len(pend_av) > 2:
                            emit_av(pend_av.popleft())
                    while pend_av:
                        emit_av(pend_av.popleft())
                    norm_q.extend(make_norm_pair(p, nb, av))
                for nch in range(NB // P):
                    for co in range(2):
                        tail_q.append(make_proj_unit(nb, nch, co))
            while norm_q:
                norm_q.popleft()()
            while tail_q:
                tail_q.popleft()()


def _get_module():
    global _cached
    if _cached is None:
        _cached = _build_module()
    return _cached


def _make_in_maps(x, context, qkv_w, kv_y_w, proj_w, q_norm_w, k_norm_w):
    import ml_dtypes
    b16 = ml_dtypes.bfloat16
    GD = HG * D  # 256 head-dims per core
    # rmsnorm weights are folded into the q/k projection weights; the
    # sumsq stationary then divides by w^2 so the variance is still that of
    # the raw projection.  (w==0 rows are zero either way.)
    qw = np.where(q_norm_w == 0, 1.0, q_norm_w).astype(np.float32)
    kw = np.where(k_norm_w == 0, 1.0, k_norm_w).astype(np.float32)
    indsum = np.zeros((P, 2, D + 1), np.float32)
    indsum[0:D, 0, 0] = 1.0 / qw ** 2
    indsum[D:P, 0, D] = 1.0 / qw ** 2
    indsum[0:D, 1, 0] = 1.0 / kw ** 2
    indsum[D:P, 1, D] = 1.0 / kw ** 2
    indsum = indsum.reshape(P, 2 * (D + 1))
    ones64 = np.ones((1, D), np.float32)
    qw_t = np.tile(q_norm_w, HG)[:, None].astype(np.float32)
    kw_t = np.tile(k_norm_w, HG)[:, None].astype(np.float32)
    projT = np.ascontiguousarray(proj_w.T)  # [ci, co]

    xT = [np.ascontiguousarray(x[b].T).astype(b16) for b in range(B)]
    cT = [np.ascontiguousarray(context[b].T).astype(b16) for b in range(B)]

    in_maps = []
    for core in range(8):
        b, hg = divmod(core, 4)
        r0 = hg * GD
        wq = qkv_w[r0:r0 + GD] * qw_t
        wk = qkv_w[DIM + r0:DIM + r0 + GD] * kw_t
        wv = qkv_w[2 * DIM + r0:2 * DIM + r0 + GD]
        wky = kv_y_w[r0:r0 + GD] * kw_t
        wvy = kv_y_w[DIM + r0:DIM + r0 + GD]
        in_maps.append({
            "xT": xT[b],
            "ctxT": cT[b],
            "wqkvT": np.ascontiguousarray(
                np.concatenate([wq, wk, wv], 0).T).astype(b16),
            "wkvyT": np.ascontiguousarray(
                np.concatenate([wky, wvy], 0).T).astype(b16),
            "wpT": np.ascontiguousarray(projT[r0:r0 + GD]).astype(b16),
            "indsum": indsum.astype(b16),
            "ones64": ones64,
            "ones64b": ones64.astype(b16),
        })
    return in_maps


def kernel(x, context, qkv_w, kv_y_w, proj_w, proj_b, q_norm_w, k_norm_w):
    x = np.asarray(x, np.float32)
    context = np.asarray(context, np.float32)
    qkv_w = np.asarray(qkv_w, np.float32)
    kv_y_w = np.asarray(kv_y_w, np.float32)
    proj_w = np.asarray(proj_w, np.float32)
    proj_b = np.asarray(proj_b, np.float32)
    q_norm_w = np.asarray(q_norm_w, np.float32)
    k_norm_w = np.asarray(k_norm_w, np.float32)

    nc = _get_module()
    in_maps = _make_in_maps(x, context, qkv_w, kv_y_w, proj_w,
                            q_norm_w, k_norm_w)
    res = run_bass_kernel_spmd(nc, in_maps, core_ids=list(range(8)))
    out = np.zeros((B, N, DIM), np.float32)
    for core in range(8):
        b = core // 4
        out[b] += res.results[core]["out"]
    out += proj_b[None, None, :]
    return out
